# revision 48
# baseline (speedup 1.0000x reference)
"""Trainium2 Bass kernel for nn_BiDenseConv2d (binarized 3x3 conv + sync-BN + channel bypass).

Shapes (hardcoded): x [8, 48, 224, 224] f32 -> out [8, 64, 224, 224] f32.
Sharding: data-parallel over batch, 1 image per core; BN stats all-reduced
([64,2] f32 AllGather); weights replicated.

Per-core pipeline (phases overlap via Tile dataflow; loads interleaved
4-ahead with the binarize chunks so no queue head-blocks):
  1. binarize, 12 half-chunks [128p=(8g+s), 7 rows x 226]: rint via the fp32
     magic constant (Pool; DVE for the last two), is_ge in-place (DVE),
     affine {1,0}->+-1 fp8 on the row interior (ACT; ring pads zeroed once).
  2. scatter to conv layout xa2f [96, 226, 226] fp8: one merged 8-seg DMA per
     chunk for the A half and one for the B half (B[r] = A[r+1], scattered
     from the same chunk).
  3. conv: fp8 DoubleRow matmuls, M=128 (low 64 out-cols = out rows y0+r,
     high 64 = y0+2+r), N=224, 10 DR per 4-row bank; 18 (rho, kh, kw) taps
     packed into (delta in {0,2,4}) x kw tiles over the 2-row stack, tile-pair
     strides >= 224 (3D non-overlapping moving APs); +-1 acts x +-1 weights
     accumulate exact integer sums in PSUM f32 ([128, 2, 512] bank-aligned
     pairs), evicted to fp16.
  4. BN: per-2-bank sums (DVE evict accum) + sumsq (ACT Square accum);
     AllGather of raw [128, 2] sums; (core, half) entries gathered onto both
     partition halves so k = gamma*s*rsqrt(s^2 var + eps), c = beta - mu*k
     (s = mean|w|) compute on 128 partitions without a broadcast.
  5. bypass: host supplies xhalf fp16 [64, H*W] (48 identity channels + 16
     group means); all 8 seg tiles prefetched into the y layout; pass 2:
     bypc = bp + c (ACT), ob = y*k + bypc (one DVE scalar_tensor_tensor per
     half-seg), stored via Pool SWDGE.

Conv input channel at slot 16c+g is channel 15c+g (g<15) / 45+c (g=15),
folded into the weights host-side. Output layout matches the baseline.
"""
import sys
import numpy as np

sys.path.insert(0, '/opt/trn_rl_repo')

B, CIN, COUT, H, W = 8, 48, 64, 224, 224
NCORES = 8
SEGS, SEGR = 8, 28
PW = 226
RQ = 14 * PW            # 3164 elems per (c, hf) chunk row-block
NBANK = 56
BN_EPS = 1e-5
MAGIC = 12582912.0

_cache = {}

SLOT_TO_CH = np.zeros(48, np.int64)
for _c in range(3):
    for _g in range(16):
        SLOT_TO_CH[16 * _c + _g] = (45 + _c) if _g == 15 else (15 * _c + _g)

# DR tile pairs (delta, kw); 'z' = zero-weight tile (arbitrary in-bounds read)
DR_PAIRS = [((0, 0), (2, 0)),
            ((0, 1), (2, 1)),
            ((0, 2), (4, 0)),
            ((2, 2), (4, 1)),
            ((1, 0, 'z'), (4, 2))]


def _build(general_affine: bool):
    from concourse import bacc, tile, mybir
    from concourse.ap import AP
    mt = mybir.dt
    AO = mybir.AluOpType
    AF = mybir.ActivationFunctionType

    nc = bacc.Bacc("TRN2", target_bir_lowering=False, debug=False,
                   num_devices=NCORES)

    xdev_d = nc.dram_tensor("xdev", [128, 2, 3, RQ], mt.float32,
                            kind="ExternalInput")
    xhalf_d = nc.dram_tensor("xhalf", [COUT, H * W], mt.float16,
                             kind="ExternalInput")
    wdr_d = nc.dram_tensor("wdr", [96, 5 * 2 * 128], mt.float8e4,
                           kind="ExternalInput")
    cst_d = nc.dram_tensor("cst", [128, 4], mt.float32, kind="ExternalInput")
    coef_d = nc.dram_tensor("coef", [128, 8], mt.float32, kind="ExternalInput")
    out_d = nc.dram_tensor("out", [2, COUT, NBANK, 448], mt.float32,
                           kind="ExternalOutput")

    with tile.TileContext(nc) as tc:
        with tc.tile_pool(name="main", bufs=1) as P, \
             tc.tile_pool(name="psum", bufs=8, space="PSUM") as PS, \
             tc.tile_pool(name="dram", bufs=1, space="DRAM") as D:

            # ---- persistent tiles ----
            xa2f = P.tile([96, PW, PW], mt.float8e4)
            y = P.tile([128, NBANK, 448], mt.float16)
            sums = P.tile([128, NBANK // 2], mt.float32)
            sqs = P.tile([128, NBANK // 2], mt.float32)

            neg1 = P.tile([128, 1], mt.float32)
            nc.vector.memset(neg1[:], -1.0)
            scr1 = P.tile([128, 1], mt.float32)
            nc.vector.memset(scr1[:], 1.0)
            nc.scalar.activation(scr1[:], scr1[:], AF.Sqrt)
            two = P.tile([128, 1], mt.float32)
            nc.vector.memset(two[:], 2.0)

            # top/bottom pads; A row 224 is rewritten by the (h1, s7) scatter
            nc.vector.memset(xa2f[0:96, 0, :], 0.0)
            nc.vector.memset(xa2f[0:96, 224:226, :], 0.0)

            # ---- binarize + scatter, half-chunks (hf, hh, c) of 7 rows ----
            # all loads issued first so the SP queue never head-blocks them
            HQ = RQ // 2            # 1582 = 7*226
            xv = xdev_d.ap().rearrange("p f c (h q) -> p f c h q", h=2)
            chunks = [(hf, hh, c) for hf in range(2) for hh in range(2)
                      for c in range(3)]
            x1s = {}

            def load_chunk(ch):
                hf, hh, c = ch
                x1h = P.tile([128, HQ], mt.float32, tag="x1", bufs=3,
                             name=f"x1_{hf}_{hh}_{c}")
                nc.sync.dma_start(x1h[:], xv[:, hf, c, hh, :])
                x1s[ch] = x1h

            for ch in chunks[:4]:
                load_chunk(ch)

            # constants (issued after the first x loads; needed much later)
            wdr = P.tile([96, 5, 2, 128], mt.float8e4)
            nc.sync.dma_start(
                wdr[:], wdr_d.ap().rearrange("p (d t m) -> p d t m", d=5, t=2))
            cst = P.tile([128, 4], mt.float32)
            nc.sync.dma_start(cst[:], cst_d.ap())
            coef = P.tile([128, 8], mt.float32)
            if general_affine:
                nc.sync.dma_start(coef[:], coef_d.ap())

            for ci, (hf, hh, c) in enumerate(chunks):
                x1h = x1s[(hf, hh, c)]
                if general_affine:
                    nc.vector.tensor_scalar(
                        x1h[:], x1h[:], coef[:, c:c + 1],
                        coef[:, 3 + c:4 + c], AO.mult, AO.add)
                m1 = P.tile([128, HQ], mt.bfloat16, tag="m1", bufs=3,
                            name=f"m1_{hf}_{hh}_{c}")
                rint_eng = nc.vector if ci >= 10 else nc.gpsimd
                rint_eng.tensor_scalar(m1[:], x1h[:], MAGIC, MAGIC,
                                       AO.add, AO.subtract)
                nc.vector.tensor_tensor(m1[:], x1h[:], m1[:], AO.is_ge)
                xa1b = P.tile([128, 7, PW], mt.float8e4, tag="xa1b", bufs=2,
                              name=f"xa1b_{hf}_{hh}_{c}")
                if ci < 2:   # ring pads zeroed once; op3 writes interior only
                    nc.gpsimd.memset(xa1b[:, :, 0], 0.0)
                    nc.gpsimd.memset(xa1b[:, :, 225], 0.0)
                m1v = m1[:].rearrange("p (a b) -> p a b", a=7)
                if ci >= 10:
                    nc.vector.tensor_scalar(xa1b[:, :, 1:225],
                                            m1v[:, :, 1:225], 2.0, 1.0,
                                            AO.mult, AO.subtract)
                else:
                    nc.scalar.activation(xa1b[:, :, 1:225], m1v[:, :, 1:225],
                                         AF.Identity, bias=neg1[:],
                                         scale=two[:])
                # scatter all 8 segs in one DMA (partition p = 8g + s)
                abase = xa2f[16 * c:16 * c + 16, 0:1, 0:1]
                dst = AP(abase.tensor,
                         int(abase.offset) + (14 * hf + 7 * hh + 1) * PW,
                         [[int(abase.ap[0][0]), 16], [28 * PW, SEGS], [1, HQ]])
                nc.sync.dma_start(dst, xa1b[:].rearrange("p a b -> p (a b)"))
                # B-half scatter: B[r] = A[r+1], same source chunk
                bbase = xa2f[48 + 16 * c:64 + 16 * c, 0:1, 0:1]
                bdst = AP(bbase.tensor,
                          int(bbase.offset) + (14 * hf + 7 * hh) * PW,
                          [[int(bbase.ap[0][0]), 16], [28 * PW, SEGS], [1, HQ]])
                nc.sync.dma_start(bdst, xa1b[:].rearrange("p a b -> p (a b)"))
                if ci + 4 < len(chunks):
                    load_chunk(chunks[ci + 4])

            # ---- conv: DoubleRow matmuls ----
            xbase = xa2f[0:96, 0:1, 0:1]
            pstride = int(xbase.ap[0][0])
            xoff = int(xbase.offset)

            perf = mybir.MatmulPerfMode.DoubleRow
            for bp2 in range(NBANK // 2):
                ps = PS.tile([128, 2, 512], mt.float32, tag="ps", bufs=4,
                             name=f"ps_{bp2}")
                for half in range(2):
                    b = 2 * bp2 + half
                    y0 = 4 * b
                    for r in range(2):
                        for d, (t0, t1) in enumerate(DR_PAIRS):
                            o0 = (y0 + r + t0[0]) * PW + t0[1]
                            o1 = (y0 + r + t1[0]) * PW + t1[1]
                            mv = AP(xbase.tensor, xoff + o0,
                                    [[pstride, 96], [o1 - o0, 2], [1, 224]])
                            nc.tensor.matmul(
                                ps[:, half, 224 * r:224 * r + 224],
                                wdr[:, d, :, :], mv,
                                start=(d == 0), stop=(d == 4),
                                perf_mode=perf)
                nc.vector.tensor_scalar(y[:, 2 * bp2:2 * bp2 + 2, :],
                                        ps[:, :, 0:448],
                                        1.0, None, AO.mult, AO.add,
                                        accum_out=sums[:, bp2:bp2 + 1])
                nc.scalar.activation(ps[:, :, 0:448], ps[:, :, 0:448],
                                     AF.Square,
                                     accum_out=sqs[:, bp2:bp2 + 1])

            # ---- bypass loads (fp16, straight into y layout) ----
            byp_tiles = {}

            def load_byp(s):
                bp = P.tile([128, 7, 448], mt.float16, tag="byp", bufs=8,
                            name=f"byp_{s}")
                for ci in range(2):
                    src = AP(xhalf_d.ap().tensor, 6272 * s + 448 * ci,
                             [[H * W, COUT], [896, 7], [1, 448]])
                    nc.sync.dma_start(bp[64 * ci:64 * ci + 64, :, :], src)
                return bp

            for s in range(SEGS):
                byp_tiles[s] = load_byp(s)

            # ---- stats + collective + BN affine (all on 128 partitions) ----
            kc = P.tile([128, 2], mt.float32)
            sums2 = P.tile([128, 2], mt.float32)
            nc.vector.reduce_sum(sums2[:, 0:1], sums[:], axis=mybir.AxisListType.X)
            nc.vector.reduce_sum(sums2[:, 1:2], sqs[:], axis=mybir.AxisListType.X)
            cbin = D.tile([128, 2], mt.float32)
            cbout = D.tile([NCORES, 128, 2], mt.float32)
            nc.scalar.dma_start(cbin[:], sums2[:])
            nc.gpsimd.collective_compute(
                "AllGather", mybir.AluOpType.bypass,
                replica_groups=[list(range(NCORES))],
                ins=[cbin.opt()], outs=[cbout.opt()])
            # gather (core, half) entries onto BOTH partition halves
            gath = P.tile([128, 2, 2 * NCORES], mt.float32)
            cbt = cbout[:].rearrange("g (h p) q -> g h p q", h=2)
            for half in range(2):
                src = AP(cbt.tensor, 0,
                         [[2, 64], [1, 2], [128, 2 * NCORES]])
                nc.sync.dma_start(gath[64 * half:64 * half + 64, :, :], src)
            mv2 = P.tile([128, 2], mt.float32)
            nc.vector.reduce_sum(mv2[:], gath[:], axis=mybir.AxisListType.X)
            nc.vector.tensor_scalar(mv2[:], mv2[:], 1.0 / float(B * H * W),
                                    None, AO.mult)

            m2t = P.tile([128, 1], mt.float32)
            nc.vector.tensor_tensor(m2t[:], mv2[:, 0:1], mv2[:, 0:1], AO.mult)
            vart = P.tile([128, 1], mt.float32)
            nc.vector.tensor_tensor(vart[:], mv2[:, 1:2], m2t[:], AO.subtract)
            t1 = P.tile([128, 1], mt.float32)
            nc.vector.tensor_tensor(t1[:], vart[:], cst[:, 0:1], AO.mult)
            nc.vector.tensor_scalar(t1[:], t1[:], BN_EPS, None, AO.add)
            sq = P.tile([128, 1], mt.float32)
            nc.scalar.activation(sq[:], t1[:], AF.Sqrt)
            rc = P.tile([128, 1], mt.float32)
            nc.vector.reciprocal(rc[:], sq[:])
            nc.vector.tensor_tensor(kc[:, 0:1], rc[:], cst[:, 1:2], AO.mult)
            mk = P.tile([128, 1], mt.float32)
            nc.vector.tensor_tensor(mk[:], mv2[:, 0:1], kc[:, 0:1], AO.mult)
            nc.vector.tensor_tensor(kc[:, 1:2], cst[:, 2:3], mk[:],
                                    AO.subtract)

            # ---- pass 2: affine + bypass + store ----
            for s in range(SEGS):
                bp = byp_tiles.pop(s)
                if s == 0:
                    nc.vector.tensor_scalar(bp[:], bp[:], kc[:, 1:2], None,
                                            AO.add)
                else:
                    nc.scalar.activation(bp[:], bp[:], AF.Identity,
                                         bias=kc[:, 1:2])
                for (j0, nj) in ((0, 4), (4, 3)):
                    ob = P.tile([128, 4, 448], mt.float32, tag="ob", bufs=3,
                                name=f"ob_{s}_{j0}")
                    nc.vector.scalar_tensor_tensor(
                        ob[:, 0:nj, :], y[:, 7 * s + j0:7 * s + j0 + nj, :],
                        kc[:, 0:1], bp[:, j0:j0 + nj, :], AO.mult, AO.add)
                    nc.gpsimd.dma_start(
                        out_d.ap()[:, :, 7 * s + j0:7 * s + j0 + nj, :],
                        ob[:, 0:nj, :])

    nc.compile()
    return nc


def _get_nc(general_affine):
    key = ("nc", general_affine, NCORES)
    if key not in _cache:
        _cache[key] = _build(general_affine)
    return _cache[key]


def _pack_weights(wt):
    """wt [64, 48, 3, 3] (+-1 * A, slot-permuted) -> [96, 5, 2, 128] f32."""
    w = np.zeros((96, 5, 2, 128), np.float32)
    covered = set()
    for d, pair in enumerate(DR_PAIRS):
        for t, tl in enumerate(pair):
            if len(tl) == 3:
                continue
            delta, kw = tl
            for stack in (0, 1):
                for half, rho in ((0, 0), (1, 2)):
                    kh = delta + stack - rho
                    if 0 <= kh <= 2 and (rho, kh, kw) not in covered:
                        covered.add((rho, kh, kw))
                        w[48 * stack:48 * stack + 48, d, t,
                          64 * half:64 * half + 64] = wt[:, :, kh, kw].T
    assert len(covered) == 18
    return w


def _host_prep(alpha, epsilon, tau, A, weight, gamma, beta):
    import ml_dtypes
    f8 = ml_dtypes.float8_e4m3

    eps_v = np.asarray(epsilon, np.float32).reshape(-1)
    tau_v = np.asarray(tau, np.float32).reshape(-1)
    A_v = np.asarray(A, np.float32).reshape(-1)
    if eps_v.size == 1:
        eps_v = np.full(CIN, eps_v[0], np.float32)
    if tau_v.size == 1:
        tau_v = np.full(CIN, tau_v[0], np.float32)
    if A_v.size == 1:
        A_v = np.full(CIN, A_v[0], np.float32)

    general = not (np.all(eps_v == 0.0) and np.all(tau_v == 1.0))

    w = np.asarray(weight, np.float32)
    scale = np.mean(np.abs(w), axis=(1, 2, 3), dtype=np.float32)
    waff = np.sign(w) * A_v[None, :, None, None]
    wperm = waff[:, SLOT_TO_CH, :, :]
    wdr = _pack_weights(wperm).reshape(96, -1).astype(f8)

    cst = np.zeros((64, 4), np.float32)
    cst[:, 0] = scale * scale
    cst[:, 1] = np.asarray(gamma, np.float32).reshape(-1) * scale
    cst[:, 2] = np.asarray(beta, np.float32).reshape(-1)
    cst = np.tile(cst, (2, 1))

    coef = np.zeros((128, 8), np.float32)
    if general:
        for p in range(128):
            g = p // 8
            for c in range(3):
                ch = 45 + c if g == 15 else 15 * c + g
                coef[p, c] = 1.0 / tau_v[ch]
                coef[p, 3 + c] = -eps_v[ch] / tau_v[ch]
    return general, wdr, cst, coef


def _make_xdev(xi):
    """xi [48, 224, 224] f32 -> [128, 2, 3, 3164] (rows padded to 226)."""
    xp = np.zeros((CIN, H, PW), np.float32)
    xp[:, :, 1:225] = xi
    xr = xp.reshape(CIN, SEGS, 2, RQ)       # [ch, seg, hf, 14*226]
    p = np.arange(128)
    g_idx, s_idx = p // 8, p % 8
    out = np.empty((128, 2, 3, RQ), np.float32)
    for c in range(3):
        ch = np.where(g_idx == 15, 45 + c, 15 * c + g_idx)
        out[:, :, c, :] = xr[ch, s_idx, :, :]
    return out


def _make_xhalf(xi):
    """xi [48, 224, 224] f32 -> [64, H*W] fp16 (identity + 16 group means)."""
    xh = np.empty((COUT, H * W), np.float16)
    xh[0:CIN] = xi.reshape(CIN, -1).astype(np.float16)
    xf = xi.reshape(CIN, -1)
    xh[48:63] = xf[0:45].reshape(3, 15, -1).mean(axis=0,
                                                 dtype=np.float32).astype(np.float16)
    xh[63] = xf[45:48].mean(axis=0, dtype=np.float32).astype(np.float16)
    return xh


def kernel(x, alpha, epsilon, tau, A, weight, gamma, beta):
    from concourse import bass_utils

    x = np.asarray(x, np.float32)
    general, wdr, cst, coef = _host_prep(alpha, epsilon, tau, A,
                                         weight, gamma, beta)
    nc = _get_nc(general)

    in_maps = []
    for i in range(NCORES):
        xi = np.ascontiguousarray(x[i])
        in_maps.append({
            "xdev": _make_xdev(xi),
            "xhalf": _make_xhalf(xi),
            "wdr": wdr, "cst": cst, "coef": coef,
        })
    res = bass_utils.run_bass_kernel_spmd(nc, in_maps,
                                          core_ids=list(range(NCORES)))
    out = np.stack([
        res.results[i]["out"].reshape(2, COUT, NBANK, 2, 224)
        .transpose(1, 2, 0, 3, 4).reshape(COUT, H, W)
        for i in range(NCORES)
    ])
    return out.astype(np.float32)


# revision 49
# speedup vs baseline: 1.0538x; 1.0538x over previous
"""Trainium2 Bass kernel for nn_BiDenseConv2d (binarized 3x3 conv + sync-BN + channel bypass).

Shapes (hardcoded): x [8, 48, 224, 224] f32 -> out [8, 64, 224, 224] f32.
Sharding: data-parallel over batch, 1 image per core; BN stats all-reduced
([64,2] f32 AllGather); weights replicated.

Per-core pipeline (phases overlap via Tile dataflow; loads interleaved
4-ahead with the binarize chunks so no queue head-blocks):
  1. binarize, 12 half-chunks [128p=(8g+s), 7 rows x 226]: rint via the fp32
     magic constant (Pool; DVE for the last two), is_ge in-place (DVE),
     affine {1,0}->+-1 fp8 on the row interior (ACT; ring pads zeroed once).
  2. scatter to conv layout xa2f [96, 226, 226] fp8: one merged 8-seg DMA per
     chunk for the A half and one for the B half (B[r] = A[r+1], scattered
     from the same chunk).
  3. conv: fp8 DoubleRow matmuls, M=128 (low 64 out-cols = out rows y0+r,
     high 64 = y0+2+r), N=224, 10 DR per 4-row bank; 18 (rho, kh, kw) taps
     packed into (delta in {0,2,4}) x kw tiles over the 2-row stack, tile-pair
     strides >= 224 (3D non-overlapping moving APs); +-1 acts x +-1 weights
     accumulate exact integer sums in PSUM f32 ([128, 2, 512] bank-aligned
     pairs), evicted to fp16.
  4. BN: per-2-bank sums (DVE evict accum) + sumsq (ACT Square accum);
     AllGather of raw [128, 2] sums; (core, half) entries gathered onto both
     partition halves so k = gamma*s*rsqrt(s^2 var + eps), c = beta - mu*k
     (s = mean|w|) compute on 128 partitions without a broadcast.
  5. bypass: host supplies xhalf fp16 [64, H*W] (48 identity channels + 16
     group means); all 8 seg tiles prefetched into the y layout; pass 2:
     bypc = bp + c (ACT), ob = y*k + bypc (one DVE scalar_tensor_tensor per
     half-seg), stored via Pool SWDGE.

Conv input channel at slot 16c+g is channel 15c+g (g<15) / 45+c (g=15),
folded into the weights host-side. Output layout matches the baseline.
"""
import sys
import numpy as np

sys.path.insert(0, '/opt/trn_rl_repo')

B, CIN, COUT, H, W = 8, 48, 64, 224, 224
NCORES = 8
SEGS, SEGR = 8, 28
PW = 226
RQ = 14 * PW            # 3164 elems per (c, hf) chunk row-block
NBANK = 56
BN_EPS = 1e-5
MAGIC = 12582912.0

_cache = {}

SLOT_TO_CH = np.zeros(48, np.int64)
for _c in range(3):
    for _g in range(16):
        SLOT_TO_CH[16 * _c + _g] = (45 + _c) if _g == 15 else (15 * _c + _g)

# DR tile pairs (delta, kw); 'z' = zero-weight tile (arbitrary in-bounds read)
DR_PAIRS = [((0, 0), (2, 0)),
            ((0, 1), (2, 1)),
            ((0, 2), (4, 0)),
            ((2, 2), (4, 1)),
            ((1, 0, 'z'), (4, 2))]


def _build(general_affine: bool):
    from concourse import bacc, tile, mybir
    from concourse.ap import AP
    mt = mybir.dt
    AO = mybir.AluOpType
    AF = mybir.ActivationFunctionType

    nc = bacc.Bacc("TRN2", target_bir_lowering=False, debug=False,
                   num_devices=NCORES)

    xdev_d = nc.dram_tensor("xdev", [128, 2, 3, RQ], mt.float32,
                            kind="ExternalInput")
    xhalf_d = nc.dram_tensor("xhalf", [COUT, H * W], mt.float16,
                             kind="ExternalInput")
    wdr_d = nc.dram_tensor("wdr", [96, 5 * 2 * 128], mt.float8e4,
                           kind="ExternalInput")
    cst_d = nc.dram_tensor("cst", [128, 4], mt.float32, kind="ExternalInput")
    coef_d = nc.dram_tensor("coef", [128, 8], mt.float32, kind="ExternalInput")
    out_d = nc.dram_tensor("out", [2, COUT, NBANK, 448], mt.float16,
                           kind="ExternalOutput")

    with tile.TileContext(nc) as tc:
        with tc.tile_pool(name="main", bufs=1) as P, \
             tc.tile_pool(name="psum", bufs=8, space="PSUM") as PS, \
             tc.tile_pool(name="dram", bufs=1, space="DRAM") as D:

            # ---- persistent tiles ----
            xa2f = P.tile([96, PW, PW], mt.float8e4)
            y = P.tile([128, NBANK, 448], mt.float16)
            sums = P.tile([128, NBANK // 2], mt.float32)
            sqs = P.tile([128, NBANK // 2], mt.float32)

            neg1 = P.tile([128, 1], mt.float32)
            nc.vector.memset(neg1[:], -1.0)
            scr1 = P.tile([128, 1], mt.float32)
            nc.vector.memset(scr1[:], 1.0)
            nc.scalar.activation(scr1[:], scr1[:], AF.Sqrt)
            two = P.tile([128, 1], mt.float32)
            nc.vector.memset(two[:], 2.0)

            # top/bottom pads; A row 224 is rewritten by the (h1, s7) scatter
            nc.vector.memset(xa2f[0:96, 0, :], 0.0)
            nc.vector.memset(xa2f[0:96, 224:226, :], 0.0)

            # ---- binarize + scatter, half-chunks (hf, hh, c) of 7 rows ----
            # all loads issued first so the SP queue never head-blocks them
            HQ = RQ // 2            # 1582 = 7*226
            xv = xdev_d.ap().rearrange("p f c (h q) -> p f c h q", h=2)
            chunks = [(hf, hh, c) for hf in range(2) for hh in range(2)
                      for c in range(3)]
            x1s = {}

            def load_chunk(ch):
                hf, hh, c = ch
                x1h = P.tile([128, HQ], mt.float32, tag="x1", bufs=3,
                             name=f"x1_{hf}_{hh}_{c}")
                nc.sync.dma_start(x1h[:], xv[:, hf, c, hh, :])
                x1s[ch] = x1h

            for ch in chunks[:4]:
                load_chunk(ch)

            # constants (issued after the first x loads; needed much later)
            wdr = P.tile([96, 5, 2, 128], mt.float8e4)
            nc.sync.dma_start(
                wdr[:], wdr_d.ap().rearrange("p (d t m) -> p d t m", d=5, t=2))
            cst = P.tile([128, 4], mt.float32)
            nc.sync.dma_start(cst[:], cst_d.ap())
            coef = P.tile([128, 8], mt.float32)
            if general_affine:
                nc.sync.dma_start(coef[:], coef_d.ap())

            for ci, (hf, hh, c) in enumerate(chunks):
                x1h = x1s[(hf, hh, c)]
                if general_affine:
                    nc.vector.tensor_scalar(
                        x1h[:], x1h[:], coef[:, c:c + 1],
                        coef[:, 3 + c:4 + c], AO.mult, AO.add)
                m1 = P.tile([128, HQ], mt.bfloat16, tag="m1", bufs=3,
                            name=f"m1_{hf}_{hh}_{c}")
                rint_eng = nc.vector if ci >= 10 else nc.gpsimd
                rint_eng.tensor_scalar(m1[:], x1h[:], MAGIC, MAGIC,
                                       AO.add, AO.subtract)
                nc.vector.tensor_tensor(m1[:], x1h[:], m1[:], AO.is_ge)
                xa1b = P.tile([128, 7, PW], mt.float8e4, tag="xa1b", bufs=2,
                              name=f"xa1b_{hf}_{hh}_{c}")
                if ci < 2:   # ring pads zeroed once; op3 writes interior only
                    nc.gpsimd.memset(xa1b[:, :, 0], 0.0)
                    nc.gpsimd.memset(xa1b[:, :, 225], 0.0)
                m1v = m1[:].rearrange("p (a b) -> p a b", a=7)
                if ci >= 10:
                    nc.vector.tensor_scalar(xa1b[:, :, 1:225],
                                            m1v[:, :, 1:225], 2.0, 1.0,
                                            AO.mult, AO.subtract)
                else:
                    nc.scalar.activation(xa1b[:, :, 1:225], m1v[:, :, 1:225],
                                         AF.Identity, bias=neg1[:],
                                         scale=two[:])
                # scatter all 8 segs in one DMA (partition p = 8g + s)
                abase = xa2f[16 * c:16 * c + 16, 0:1, 0:1]
                dst = AP(abase.tensor,
                         int(abase.offset) + (14 * hf + 7 * hh + 1) * PW,
                         [[int(abase.ap[0][0]), 16], [28 * PW, SEGS], [1, HQ]])
                nc.sync.dma_start(dst, xa1b[:].rearrange("p a b -> p (a b)"))
                # B-half scatter: B[r] = A[r+1], same source chunk
                bbase = xa2f[48 + 16 * c:64 + 16 * c, 0:1, 0:1]
                bdst = AP(bbase.tensor,
                          int(bbase.offset) + (14 * hf + 7 * hh) * PW,
                          [[int(bbase.ap[0][0]), 16], [28 * PW, SEGS], [1, HQ]])
                nc.sync.dma_start(bdst, xa1b[:].rearrange("p a b -> p (a b)"))
                if ci + 4 < len(chunks):
                    load_chunk(chunks[ci + 4])

            # ---- conv: DoubleRow matmuls ----
            xbase = xa2f[0:96, 0:1, 0:1]
            pstride = int(xbase.ap[0][0])
            xoff = int(xbase.offset)

            perf = mybir.MatmulPerfMode.DoubleRow
            for bp2 in range(NBANK // 2):
                ps = PS.tile([128, 2, 512], mt.float32, tag="ps", bufs=4,
                             name=f"ps_{bp2}")
                for half in range(2):
                    b = 2 * bp2 + half
                    y0 = 4 * b
                    for r in range(2):
                        for d, (t0, t1) in enumerate(DR_PAIRS):
                            o0 = (y0 + r + t0[0]) * PW + t0[1]
                            o1 = (y0 + r + t1[0]) * PW + t1[1]
                            mv = AP(xbase.tensor, xoff + o0,
                                    [[pstride, 96], [o1 - o0, 2], [1, 224]])
                            nc.tensor.matmul(
                                ps[:, half, 224 * r:224 * r + 224],
                                wdr[:, d, :, :], mv,
                                start=(d == 0), stop=(d == 4),
                                perf_mode=perf)
                nc.vector.tensor_scalar(y[:, 2 * bp2:2 * bp2 + 2, :],
                                        ps[:, :, 0:448],
                                        1.0, None, AO.mult, AO.add,
                                        accum_out=sums[:, bp2:bp2 + 1])
                nc.scalar.activation(ps[:, :, 0:448], ps[:, :, 0:448],
                                     AF.Square,
                                     accum_out=sqs[:, bp2:bp2 + 1])

            # ---- bypass loads (fp16, straight into y layout) ----
            byp_tiles = {}

            def load_byp(s):
                bp = P.tile([128, 7, 448], mt.float16, tag="byp", bufs=8,
                            name=f"byp_{s}")
                for ci in range(2):
                    src = AP(xhalf_d.ap().tensor, 6272 * s + 448 * ci,
                             [[H * W, COUT], [896, 7], [1, 448]])
                    nc.sync.dma_start(bp[64 * ci:64 * ci + 64, :, :], src)
                return bp

            for s in range(SEGS):
                byp_tiles[s] = load_byp(s)

            # ---- stats + collective + BN affine (all on 128 partitions) ----
            kc = P.tile([128, 2], mt.float32)
            sums2 = P.tile([128, 2], mt.float32)
            nc.vector.reduce_sum(sums2[:, 0:1], sums[:], axis=mybir.AxisListType.X)
            nc.vector.reduce_sum(sums2[:, 1:2], sqs[:], axis=mybir.AxisListType.X)
            cbin = D.tile([128, 2], mt.float32)
            cbout = D.tile([NCORES, 128, 2], mt.float32)
            nc.scalar.dma_start(cbin[:], sums2[:])
            nc.gpsimd.collective_compute(
                "AllGather", mybir.AluOpType.bypass,
                replica_groups=[list(range(NCORES))],
                ins=[cbin.opt()], outs=[cbout.opt()])
            # gather (core, half) entries onto BOTH partition halves
            gath = P.tile([128, 2, 2 * NCORES], mt.float32)
            cbt = cbout[:].rearrange("g (h p) q -> g h p q", h=2)
            for half in range(2):
                src = AP(cbt.tensor, 0,
                         [[2, 64], [1, 2], [128, 2 * NCORES]])
                nc.sync.dma_start(gath[64 * half:64 * half + 64, :, :], src)
            mv2 = P.tile([128, 2], mt.float32)
            nc.vector.reduce_sum(mv2[:], gath[:], axis=mybir.AxisListType.X)
            nc.vector.tensor_scalar(mv2[:], mv2[:], 1.0 / float(B * H * W),
                                    None, AO.mult)

            m2t = P.tile([128, 1], mt.float32)
            nc.vector.tensor_tensor(m2t[:], mv2[:, 0:1], mv2[:, 0:1], AO.mult)
            vart = P.tile([128, 1], mt.float32)
            nc.vector.tensor_tensor(vart[:], mv2[:, 1:2], m2t[:], AO.subtract)
            t1 = P.tile([128, 1], mt.float32)
            nc.vector.tensor_tensor(t1[:], vart[:], cst[:, 0:1], AO.mult)
            nc.vector.tensor_scalar(t1[:], t1[:], BN_EPS, None, AO.add)
            sq = P.tile([128, 1], mt.float32)
            nc.scalar.activation(sq[:], t1[:], AF.Sqrt)
            rc = P.tile([128, 1], mt.float32)
            nc.vector.reciprocal(rc[:], sq[:])
            nc.vector.tensor_tensor(kc[:, 0:1], rc[:], cst[:, 1:2], AO.mult)
            mk = P.tile([128, 1], mt.float32)
            nc.vector.tensor_tensor(mk[:], mv2[:, 0:1], kc[:, 0:1], AO.mult)
            nc.vector.tensor_tensor(kc[:, 1:2], cst[:, 2:3], mk[:],
                                    AO.subtract)

            # ---- pass 2: affine + bypass + store ----
            for s in range(SEGS):
                bp = byp_tiles.pop(s)
                if s == 0:
                    nc.vector.tensor_scalar(bp[:], bp[:], kc[:, 1:2], None,
                                            AO.add)
                else:
                    nc.scalar.activation(bp[:], bp[:], AF.Identity,
                                         bias=kc[:, 1:2])
                for (j0, nj) in ((0, 4), (4, 3)):
                    ob = P.tile([128, 4, 448], mt.float16, tag="ob", bufs=3,
                                name=f"ob_{s}_{j0}")
                    nc.vector.scalar_tensor_tensor(
                        ob[:, 0:nj, :], y[:, 7 * s + j0:7 * s + j0 + nj, :],
                        kc[:, 0:1], bp[:, j0:j0 + nj, :], AO.mult, AO.add)
                    nc.gpsimd.dma_start(
                        out_d.ap()[:, :, 7 * s + j0:7 * s + j0 + nj, :],
                        ob[:, 0:nj, :])

    nc.compile()
    return nc


def _get_nc(general_affine):
    key = ("nc", general_affine, NCORES)
    if key not in _cache:
        _cache[key] = _build(general_affine)
    return _cache[key]


def _pack_weights(wt):
    """wt [64, 48, 3, 3] (+-1 * A, slot-permuted) -> [96, 5, 2, 128] f32."""
    w = np.zeros((96, 5, 2, 128), np.float32)
    covered = set()
    for d, pair in enumerate(DR_PAIRS):
        for t, tl in enumerate(pair):
            if len(tl) == 3:
                continue
            delta, kw = tl
            for stack in (0, 1):
                for half, rho in ((0, 0), (1, 2)):
                    kh = delta + stack - rho
                    if 0 <= kh <= 2 and (rho, kh, kw) not in covered:
                        covered.add((rho, kh, kw))
                        w[48 * stack:48 * stack + 48, d, t,
                          64 * half:64 * half + 64] = wt[:, :, kh, kw].T
    assert len(covered) == 18
    return w


def _host_prep(alpha, epsilon, tau, A, weight, gamma, beta):
    import ml_dtypes
    f8 = ml_dtypes.float8_e4m3

    eps_v = np.asarray(epsilon, np.float32).reshape(-1)
    tau_v = np.asarray(tau, np.float32).reshape(-1)
    A_v = np.asarray(A, np.float32).reshape(-1)
    if eps_v.size == 1:
        eps_v = np.full(CIN, eps_v[0], np.float32)
    if tau_v.size == 1:
        tau_v = np.full(CIN, tau_v[0], np.float32)
    if A_v.size == 1:
        A_v = np.full(CIN, A_v[0], np.float32)

    general = not (np.all(eps_v == 0.0) and np.all(tau_v == 1.0))

    w = np.asarray(weight, np.float32)
    scale = np.mean(np.abs(w), axis=(1, 2, 3), dtype=np.float32)
    waff = np.sign(w) * A_v[None, :, None, None]
    wperm = waff[:, SLOT_TO_CH, :, :]
    wdr = _pack_weights(wperm).reshape(96, -1).astype(f8)

    cst = np.zeros((64, 4), np.float32)
    cst[:, 0] = scale * scale
    cst[:, 1] = np.asarray(gamma, np.float32).reshape(-1) * scale
    cst[:, 2] = np.asarray(beta, np.float32).reshape(-1)
    cst = np.tile(cst, (2, 1))

    coef = np.zeros((128, 8), np.float32)
    if general:
        for p in range(128):
            g = p // 8
            for c in range(3):
                ch = 45 + c if g == 15 else 15 * c + g
                coef[p, c] = 1.0 / tau_v[ch]
                coef[p, 3 + c] = -eps_v[ch] / tau_v[ch]
    return general, wdr, cst, coef


def _make_xdev(xi):
    """xi [48, 224, 224] f32 -> [128, 2, 3, 3164] (rows padded to 226)."""
    xp = np.zeros((CIN, H, PW), np.float32)
    xp[:, :, 1:225] = xi
    xr = xp.reshape(CIN, SEGS, 2, RQ)       # [ch, seg, hf, 14*226]
    p = np.arange(128)
    g_idx, s_idx = p // 8, p % 8
    out = np.empty((128, 2, 3, RQ), np.float32)
    for c in range(3):
        ch = np.where(g_idx == 15, 45 + c, 15 * c + g_idx)
        out[:, :, c, :] = xr[ch, s_idx, :, :]
    return out


def _make_xhalf(xi):
    """xi [48, 224, 224] f32 -> [64, H*W] fp16 (identity + 16 group means)."""
    xh = np.empty((COUT, H * W), np.float16)
    xh[0:CIN] = xi.reshape(CIN, -1).astype(np.float16)
    xf = xi.reshape(CIN, -1)
    xh[48:63] = xf[0:45].reshape(3, 15, -1).mean(axis=0,
                                                 dtype=np.float32).astype(np.float16)
    xh[63] = xf[45:48].mean(axis=0, dtype=np.float32).astype(np.float16)
    return xh


def kernel(x, alpha, epsilon, tau, A, weight, gamma, beta):
    from concourse import bass_utils

    x = np.asarray(x, np.float32)
    general, wdr, cst, coef = _host_prep(alpha, epsilon, tau, A,
                                         weight, gamma, beta)
    nc = _get_nc(general)

    in_maps = []
    for i in range(NCORES):
        xi = np.ascontiguousarray(x[i])
        in_maps.append({
            "xdev": _make_xdev(xi),
            "xhalf": _make_xhalf(xi),
            "wdr": wdr, "cst": cst, "coef": coef,
        })
    res = bass_utils.run_bass_kernel_spmd(nc, in_maps,
                                          core_ids=list(range(NCORES)))
    out = np.stack([
        res.results[i]["out"].reshape(2, COUT, NBANK, 2, 224)
        .transpose(1, 2, 0, 3, 4).reshape(COUT, H, W)
        for i in range(NCORES)
    ])
    return out.astype(np.float32)


# revision 50
# speedup vs baseline: 1.0706x; 1.0160x over previous
"""Trainium2 Bass kernel for nn_BiDenseConv2d (binarized 3x3 conv + sync-BN + channel bypass).

Shapes (hardcoded): x [8, 48, 224, 224] f32 -> out [8, 64, 224, 224] f32.
Sharding: data-parallel over batch, 1 image per core; BN stats all-reduced
([64,2] f32 AllGather); weights replicated.

Per-core pipeline (phases overlap via Tile dataflow; loads interleaved
4-ahead with the binarize chunks so no queue head-blocks):
  1. binarize, 12 half-chunks [128p=(8g+s), 7 rows x 226]: rint via the fp32
     magic constant (Pool; DVE for the last two), is_ge in-place (DVE),
     affine {1,0}->+-1 fp8 on the row interior (ACT; ring pads zeroed once).
  2. scatter to conv layout xa2f [96, 226, 226] fp8: one merged 8-seg DMA per
     chunk for the A half and one for the B half (B[r] = A[r+1], scattered
     from the same chunk).
  3. conv: fp8 DoubleRow matmuls, M=128 (low 64 out-cols = out rows y0+r,
     high 64 = y0+2+r), N=224, 10 DR per 4-row bank; 18 (rho, kh, kw) taps
     packed into (delta in {0,2,4}) x kw tiles over the 2-row stack, tile-pair
     strides >= 224 (3D non-overlapping moving APs); +-1 acts x +-1 weights
     accumulate exact integer sums in PSUM f32 ([128, 2, 512] bank-aligned
     pairs), evicted to fp16.
  4. BN: per-2-bank sums (DVE evict accum) + sumsq (ACT Square accum);
     AllGather of raw [128, 2] sums; (core, half) entries gathered onto both
     partition halves so k = gamma*s*rsqrt(s^2 var + eps), c = beta - mu*k
     (s = mean|w|) compute on 128 partitions without a broadcast.
  5. bypass: host supplies xhalf fp16 [64, H*W] (48 identity channels + 16
     group means); all 8 seg tiles prefetched into the y layout; pass 2:
     bypc = bp + c (ACT), ob = y*k + bypc (one DVE scalar_tensor_tensor per
     half-seg), stored via Pool SWDGE.

Conv input channel at slot 16c+g is channel 15c+g (g<15) / 45+c (g=15),
folded into the weights host-side. Output layout matches the baseline.
"""
import sys
import numpy as np

sys.path.insert(0, '/opt/trn_rl_repo')

B, CIN, COUT, H, W = 8, 48, 64, 224, 224
NCORES = 8
SEGS, SEGR = 8, 28
PW = 226
RQ = 14 * PW            # 3164 elems per (c, hf) chunk row-block
NBANK = 56
BN_EPS = 1e-5
MAGIC = 12582912.0

_cache = {}

SLOT_TO_CH = np.zeros(48, np.int64)
for _c in range(3):
    for _g in range(16):
        SLOT_TO_CH[16 * _c + _g] = (45 + _c) if _g == 15 else (15 * _c + _g)

# DR tile pairs (delta, kw); 'z' = zero-weight tile (arbitrary in-bounds read)
DR_PAIRS = [((0, 0), (2, 0)),
            ((0, 1), (2, 1)),
            ((0, 2), (4, 0)),
            ((2, 2), (4, 1)),
            ((1, 0, 'z'), (4, 2))]


def _build(general_affine: bool):
    from concourse import bacc, tile, mybir
    from concourse.ap import AP
    mt = mybir.dt
    AO = mybir.AluOpType
    AF = mybir.ActivationFunctionType

    nc = bacc.Bacc("TRN2", target_bir_lowering=False, debug=False,
                   num_devices=NCORES)

    xdev_d = nc.dram_tensor("xdev", [128, 2, 3, RQ], mt.float32,
                            kind="ExternalInput")
    xhalf_d = nc.dram_tensor("xhalf", [COUT, H * W], mt.float16,
                             kind="ExternalInput")
    wdr_d = nc.dram_tensor("wdr", [96, 5 * 2 * 128], mt.float8e4,
                           kind="ExternalInput")
    cst_d = nc.dram_tensor("cst", [128, 4], mt.float32, kind="ExternalInput")
    coef_d = nc.dram_tensor("coef", [128, 8], mt.float32, kind="ExternalInput")
    out_d = nc.dram_tensor("out", [2, COUT, NBANK, 448], mt.float16,
                           kind="ExternalOutput")

    with tile.TileContext(nc) as tc:
        with tc.tile_pool(name="main", bufs=1) as P, \
             tc.tile_pool(name="psum", bufs=8, space="PSUM") as PS, \
             tc.tile_pool(name="dram", bufs=1, space="DRAM") as D:

            # ---- persistent tiles ----
            xa2f = P.tile([96, PW, PW], mt.float8e4)
            y = P.tile([128, NBANK, 448], mt.float16)
            sums = P.tile([128, NBANK // 2], mt.float32)
            sqs = P.tile([128, NBANK // 2], mt.float32)

            neg1 = P.tile([128, 1], mt.float32)
            nc.vector.memset(neg1[:], -1.0)
            scr1 = P.tile([128, 1], mt.float32)
            nc.vector.memset(scr1[:], 1.0)
            nc.scalar.activation(scr1[:], scr1[:], AF.Sqrt)
            two = P.tile([128, 1], mt.float32)
            nc.vector.memset(two[:], 2.0)

            # top/bottom pads; A row 224 is rewritten by the (h1, s7) scatter
            nc.vector.memset(xa2f[0:96, 0, :], 0.0)
            nc.vector.memset(xa2f[0:96, 224:226, :], 0.0)

            # ---- binarize + scatter, half-chunks (hf, hh, c) of 7 rows ----
            # all loads issued first so the SP queue never head-blocks them
            HQ = RQ // 2            # 1582 = 7*226
            xv = xdev_d.ap().rearrange("p f c (h q) -> p f c h q", h=2)
            chunks = [(hf, hh, c) for hf in range(2) for hh in range(2)
                      for c in range(3)]
            x1s = {}

            def load_chunk(ch):
                hf, hh, c = ch
                x1h = P.tile([128, HQ], mt.float32, tag="x1", bufs=3,
                             name=f"x1_{hf}_{hh}_{c}")
                nc.sync.dma_start(x1h[:], xv[:, hf, c, hh, :])
                x1s[ch] = x1h

            for ch in chunks[:4]:
                load_chunk(ch)

            # constants (issued after the first x loads; needed much later)
            wdr = P.tile([96, 5, 2, 128], mt.float8e4)
            nc.sync.dma_start(
                wdr[:], wdr_d.ap().rearrange("p (d t m) -> p d t m", d=5, t=2))
            cst = P.tile([128, 4], mt.float32)
            nc.sync.dma_start(cst[:], cst_d.ap())
            coef = P.tile([128, 8], mt.float32)
            if general_affine:
                nc.sync.dma_start(coef[:], coef_d.ap())

            for ci, (hf, hh, c) in enumerate(chunks):
                x1h = x1s[(hf, hh, c)]
                if general_affine:
                    nc.vector.tensor_scalar(
                        x1h[:], x1h[:], coef[:, c:c + 1],
                        coef[:, 3 + c:4 + c], AO.mult, AO.add)
                m1 = P.tile([128, HQ], mt.bfloat16, tag="m1", bufs=3,
                            name=f"m1_{hf}_{hh}_{c}")
                rint_eng = nc.vector if ci >= 10 else nc.gpsimd
                rint_eng.tensor_scalar(m1[:], x1h[:], MAGIC, MAGIC,
                                       AO.add, AO.subtract)
                nc.vector.tensor_tensor(m1[:], x1h[:], m1[:], AO.is_ge)
                xa1b = P.tile([128, 7, PW], mt.float8e4, tag="xa1b", bufs=2,
                              name=f"xa1b_{hf}_{hh}_{c}")
                if ci < 2:   # ring pads zeroed once; op3 writes interior only
                    nc.gpsimd.memset(xa1b[:, :, 0], 0.0)
                    nc.gpsimd.memset(xa1b[:, :, 225], 0.0)
                m1v = m1[:].rearrange("p (a b) -> p a b", a=7)
                if ci >= 10:
                    nc.vector.tensor_scalar(xa1b[:, :, 1:225],
                                            m1v[:, :, 1:225], 2.0, 1.0,
                                            AO.mult, AO.subtract)
                else:
                    nc.scalar.activation(xa1b[:, :, 1:225], m1v[:, :, 1:225],
                                         AF.Identity, bias=neg1[:],
                                         scale=two[:])
                # scatter all 8 segs in one DMA (partition p = 8g + s)
                abase = xa2f[16 * c:16 * c + 16, 0:1, 0:1]
                dst = AP(abase.tensor,
                         int(abase.offset) + (14 * hf + 7 * hh + 1) * PW,
                         [[int(abase.ap[0][0]), 16], [28 * PW, SEGS], [1, HQ]])
                nc.sync.dma_start(dst, xa1b[:].rearrange("p a b -> p (a b)"))
                # B-half scatter: B[r] = A[r+1], same source chunk
                bbase = xa2f[48 + 16 * c:64 + 16 * c, 0:1, 0:1]
                bdst = AP(bbase.tensor,
                          int(bbase.offset) + (14 * hf + 7 * hh) * PW,
                          [[int(bbase.ap[0][0]), 16], [28 * PW, SEGS], [1, HQ]])
                nc.sync.dma_start(bdst, xa1b[:].rearrange("p a b -> p (a b)"))
                if ci + 4 < len(chunks):
                    load_chunk(chunks[ci + 4])

            # ---- conv: DoubleRow matmuls ----
            xbase = xa2f[0:96, 0:1, 0:1]
            pstride = int(xbase.ap[0][0])
            xoff = int(xbase.offset)

            perf = mybir.MatmulPerfMode.DoubleRow
            for bp2 in range(NBANK // 2):
                ps = PS.tile([128, 2, 512], mt.float32, tag="ps", bufs=4,
                             name=f"ps_{bp2}")
                for half in range(2):
                    b = 2 * bp2 + half
                    y0 = 4 * b
                    for r in range(2):
                        for d, (t0, t1) in enumerate(DR_PAIRS):
                            o0 = (y0 + r + t0[0]) * PW + t0[1]
                            o1 = (y0 + r + t1[0]) * PW + t1[1]
                            mv = AP(xbase.tensor, xoff + o0,
                                    [[pstride, 96], [o1 - o0, 2], [1, 224]])
                            nc.tensor.matmul(
                                ps[:, half, 224 * r:224 * r + 224],
                                wdr[:, d, :, :], mv,
                                start=(d == 0), stop=(d == 4),
                                perf_mode=perf)
                nc.vector.tensor_scalar(y[:, 2 * bp2:2 * bp2 + 2, :],
                                        ps[:, :, 0:448],
                                        1.0, None, AO.mult, AO.add,
                                        accum_out=sums[:, bp2:bp2 + 1])
                nc.scalar.activation(ps[:, :, 0:448], ps[:, :, 0:448],
                                     AF.Square,
                                     accum_out=sqs[:, bp2:bp2 + 1])

            # ---- bypass loads (fp16, straight into y layout) ----
            byp_tiles = {}

            def load_byp(s):
                bp = P.tile([128, 7, 448], mt.float16, tag="byp", bufs=8,
                            name=f"byp_{s}")
                for ci in range(2):
                    src = AP(xhalf_d.ap().tensor, 6272 * s + 448 * ci,
                             [[H * W, COUT], [896, 7], [1, 448]])
                    nc.sync.dma_start(bp[64 * ci:64 * ci + 64, :, :], src)
                return bp

            for s in range(SEGS):
                byp_tiles[s] = load_byp(s)

            # ---- stats + collective + BN affine (all on 128 partitions) ----
            kc = P.tile([128, 2], mt.float32)
            sums2 = P.tile([128, 2], mt.float32)
            nc.vector.reduce_sum(sums2[:, 0:1], sums[:], axis=mybir.AxisListType.X)
            nc.vector.reduce_sum(sums2[:, 1:2], sqs[:], axis=mybir.AxisListType.X)
            cbin = D.tile([128, 2], mt.float32)
            cbout = D.tile([NCORES, 128, 2], mt.float32)
            nc.scalar.dma_start(cbin[:], sums2[:])
            nc.gpsimd.collective_compute(
                "AllGather", mybir.AluOpType.bypass,
                replica_groups=[list(range(NCORES))],
                ins=[cbin.opt()], outs=[cbout.opt()])
            # gather (core, half) entries onto BOTH partition halves
            gath = P.tile([128, 2, 2 * NCORES], mt.float32)
            cbt = cbout[:].rearrange("g (h p) q -> g h p q", h=2)
            for half in range(2):
                src = AP(cbt.tensor, 0,
                         [[2, 64], [1, 2], [128, 2 * NCORES]])
                nc.sync.dma_start(gath[64 * half:64 * half + 64, :, :], src)
            mv2 = P.tile([128, 2], mt.float32)
            nc.vector.reduce_sum(mv2[:], gath[:], axis=mybir.AxisListType.X)
            nc.vector.tensor_scalar(mv2[:], mv2[:], 1.0 / float(B * H * W),
                                    None, AO.mult)

            m2t = P.tile([128, 1], mt.float32)
            nc.vector.tensor_tensor(m2t[:], mv2[:, 0:1], mv2[:, 0:1], AO.mult)
            vart = P.tile([128, 1], mt.float32)
            nc.vector.tensor_tensor(vart[:], mv2[:, 1:2], m2t[:], AO.subtract)
            t1 = P.tile([128, 1], mt.float32)
            nc.vector.tensor_tensor(t1[:], vart[:], cst[:, 0:1], AO.mult)
            nc.vector.tensor_scalar(t1[:], t1[:], BN_EPS, None, AO.add)
            sq = P.tile([128, 1], mt.float32)
            nc.scalar.activation(sq[:], t1[:], AF.Sqrt)
            rc = P.tile([128, 1], mt.float32)
            nc.vector.reciprocal(rc[:], sq[:])
            nc.vector.tensor_tensor(kc[:, 0:1], rc[:], cst[:, 1:2], AO.mult)
            mk = P.tile([128, 1], mt.float32)
            nc.vector.tensor_tensor(mk[:], mv2[:, 0:1], kc[:, 0:1], AO.mult)
            nc.vector.tensor_tensor(kc[:, 1:2], cst[:, 2:3], mk[:],
                                    AO.subtract)

            # ---- pass 2: affine + bypass + store ----
            for s in range(SEGS):
                bp = byp_tiles.pop(s)
                if s == 0:
                    nc.vector.tensor_scalar(bp[:], bp[:], kc[:, 1:2], None,
                                            AO.add)
                else:
                    nc.scalar.activation(bp[:], bp[:], AF.Identity,
                                         bias=kc[:, 1:2])
                ob = P.tile([128, 7, 448], mt.float16, tag="ob", bufs=3,
                            name=f"ob_{s}")
                nc.vector.scalar_tensor_tensor(
                    ob[:], y[:, 7 * s:7 * s + 7, :],
                    kc[:, 0:1], bp[:], AO.mult, AO.add)
                nc.gpsimd.dma_start(
                    out_d.ap()[:, :, 7 * s:7 * s + 7, :], ob[:])

    nc.compile()
    return nc


def _get_nc(general_affine):
    key = ("nc", general_affine, NCORES)
    if key not in _cache:
        _cache[key] = _build(general_affine)
    return _cache[key]


def _pack_weights(wt):
    """wt [64, 48, 3, 3] (+-1 * A, slot-permuted) -> [96, 5, 2, 128] f32."""
    w = np.zeros((96, 5, 2, 128), np.float32)
    covered = set()
    for d, pair in enumerate(DR_PAIRS):
        for t, tl in enumerate(pair):
            if len(tl) == 3:
                continue
            delta, kw = tl
            for stack in (0, 1):
                for half, rho in ((0, 0), (1, 2)):
                    kh = delta + stack - rho
                    if 0 <= kh <= 2 and (rho, kh, kw) not in covered:
                        covered.add((rho, kh, kw))
                        w[48 * stack:48 * stack + 48, d, t,
                          64 * half:64 * half + 64] = wt[:, :, kh, kw].T
    assert len(covered) == 18
    return w


def _host_prep(alpha, epsilon, tau, A, weight, gamma, beta):
    import ml_dtypes
    f8 = ml_dtypes.float8_e4m3

    eps_v = np.asarray(epsilon, np.float32).reshape(-1)
    tau_v = np.asarray(tau, np.float32).reshape(-1)
    A_v = np.asarray(A, np.float32).reshape(-1)
    if eps_v.size == 1:
        eps_v = np.full(CIN, eps_v[0], np.float32)
    if tau_v.size == 1:
        tau_v = np.full(CIN, tau_v[0], np.float32)
    if A_v.size == 1:
        A_v = np.full(CIN, A_v[0], np.float32)

    general = not (np.all(eps_v == 0.0) and np.all(tau_v == 1.0))

    w = np.asarray(weight, np.float32)
    scale = np.mean(np.abs(w), axis=(1, 2, 3), dtype=np.float32)
    waff = np.sign(w) * A_v[None, :, None, None]
    wperm = waff[:, SLOT_TO_CH, :, :]
    wdr = _pack_weights(wperm).reshape(96, -1).astype(f8)

    cst = np.zeros((64, 4), np.float32)
    cst[:, 0] = scale * scale
    cst[:, 1] = np.asarray(gamma, np.float32).reshape(-1) * scale
    cst[:, 2] = np.asarray(beta, np.float32).reshape(-1)
    cst = np.tile(cst, (2, 1))

    coef = np.zeros((128, 8), np.float32)
    if general:
        for p in range(128):
            g = p // 8
            for c in range(3):
                ch = 45 + c if g == 15 else 15 * c + g
                coef[p, c] = 1.0 / tau_v[ch]
                coef[p, 3 + c] = -eps_v[ch] / tau_v[ch]
    return general, wdr, cst, coef


def _make_xdev(xi):
    """xi [48, 224, 224] f32 -> [128, 2, 3, 3164] (rows padded to 226)."""
    xp = np.zeros((CIN, H, PW), np.float32)
    xp[:, :, 1:225] = xi
    xr = xp.reshape(CIN, SEGS, 2, RQ)       # [ch, seg, hf, 14*226]
    p = np.arange(128)
    g_idx, s_idx = p // 8, p % 8
    out = np.empty((128, 2, 3, RQ), np.float32)
    for c in range(3):
        ch = np.where(g_idx == 15, 45 + c, 15 * c + g_idx)
        out[:, :, c, :] = xr[ch, s_idx, :, :]
    return out


def _make_xhalf(xi):
    """xi [48, 224, 224] f32 -> [64, H*W] fp16 (identity + 16 group means)."""
    xh = np.empty((COUT, H * W), np.float16)
    xh[0:CIN] = xi.reshape(CIN, -1).astype(np.float16)
    xf = xi.reshape(CIN, -1)
    xh[48:63] = xf[0:45].reshape(3, 15, -1).mean(axis=0,
                                                 dtype=np.float32).astype(np.float16)
    xh[63] = xf[45:48].mean(axis=0, dtype=np.float32).astype(np.float16)
    return xh


def kernel(x, alpha, epsilon, tau, A, weight, gamma, beta):
    from concourse import bass_utils

    x = np.asarray(x, np.float32)
    general, wdr, cst, coef = _host_prep(alpha, epsilon, tau, A,
                                         weight, gamma, beta)
    nc = _get_nc(general)

    in_maps = []
    for i in range(NCORES):
        xi = np.ascontiguousarray(x[i])
        in_maps.append({
            "xdev": _make_xdev(xi),
            "xhalf": _make_xhalf(xi),
            "wdr": wdr, "cst": cst, "coef": coef,
        })
    res = bass_utils.run_bass_kernel_spmd(nc, in_maps,
                                          core_ids=list(range(NCORES)))
    out = np.stack([
        res.results[i]["out"].reshape(2, COUT, NBANK, 2, 224)
        .transpose(1, 2, 0, 3, 4).reshape(COUT, H, W)
        for i in range(NCORES)
    ])
    return out.astype(np.float32)


# revision 53
# speedup vs baseline: 1.0735x; 1.0027x over previous
"""Trainium2 Bass kernel for nn_BiDenseConv2d (binarized 3x3 conv + sync-BN + channel bypass).

Shapes (hardcoded): x [8, 48, 224, 224] f32 -> out [8, 64, 224, 224] f32.
Sharding: data-parallel over batch, 1 image per core; BN stats all-reduced
([64,2] f32 AllGather); weights replicated.

Per-core pipeline (phases overlap via Tile dataflow; loads interleaved
4-ahead with the binarize chunks so no queue head-blocks):
  1. binarize, 12 half-chunks [128p=(8g+s), 7 rows x 226]: rint via the fp32
     magic constant (Pool; DVE for the last two), is_ge in-place (DVE),
     affine {1,0}->+-1 fp8 on the row interior (ACT; ring pads zeroed once).
  2. scatter to conv layout xa2f [96, 226, 226] fp8: one merged 8-seg DMA per
     chunk for the A half and one for the B half (B[r] = A[r+1], scattered
     from the same chunk).
  3. conv: fp8 DoubleRow matmuls, M=128 (low 64 out-cols = out rows y0+r,
     high 64 = y0+2+r), N=224, 10 DR per 4-row bank; 18 (rho, kh, kw) taps
     packed into (delta in {0,2,4}) x kw tiles over the 2-row stack, tile-pair
     strides >= 224 (3D non-overlapping moving APs); +-1 acts x +-1 weights
     accumulate exact integer sums in PSUM f32 ([128, 2, 512] bank-aligned
     pairs), evicted to fp16.
  4. BN: per-2-bank sums (DVE evict accum) + sumsq (ACT Square accum);
     AllGather of raw [128, 2] sums; (core, half) entries gathered onto both
     partition halves so k = gamma*s*rsqrt(s^2 var + eps), c = beta - mu*k
     (s = mean|w|) compute on 128 partitions without a broadcast.
  5. bypass: host supplies xhalf fp16 [64, H*W] (48 identity channels + 16
     group means); all 8 seg tiles prefetched into the y layout; pass 2:
     bypc = bp + c (ACT), ob = y*k + bypc (one DVE scalar_tensor_tensor per
     half-seg), stored via Pool SWDGE.

Conv input channel at slot 16c+g is channel 15c+g (g<15) / 45+c (g=15),
folded into the weights host-side. Output layout matches the baseline.
"""
import sys
import numpy as np

sys.path.insert(0, '/opt/trn_rl_repo')

B, CIN, COUT, H, W = 8, 48, 64, 224, 224
NCORES = 8
SEGS, SEGR = 8, 28
PW = 226
RQ = 14 * PW            # 3164 elems per (c, hf) chunk row-block
NBANK = 56
BN_EPS = 1e-5
MAGIC = 12582912.0

_cache = {}

SLOT_TO_CH = np.zeros(48, np.int64)
for _c in range(3):
    for _g in range(16):
        SLOT_TO_CH[16 * _c + _g] = (45 + _c) if _g == 15 else (15 * _c + _g)

# DR tile pairs (delta, kw); 'z' = zero-weight tile (arbitrary in-bounds read)
DR_PAIRS = [((0, 0), (2, 0)),
            ((0, 1), (2, 1)),
            ((0, 2), (4, 0)),
            ((2, 2), (4, 1)),
            ((1, 0, 'z'), (4, 2))]


def _build(general_affine: bool):
    from concourse import bacc, tile, mybir
    from concourse.ap import AP
    mt = mybir.dt
    AO = mybir.AluOpType
    AF = mybir.ActivationFunctionType

    nc = bacc.Bacc("TRN2", target_bir_lowering=False, debug=False,
                   num_devices=NCORES)

    xdev_d = nc.dram_tensor("xdev", [128, 2, 3, RQ], mt.float32,
                            kind="ExternalInput")
    xhalf_d = nc.dram_tensor("xhalf", [COUT, H * W], mt.float16,
                             kind="ExternalInput")
    wdr_d = nc.dram_tensor("wdr", [96, 5 * 2 * 128], mt.float8e4,
                           kind="ExternalInput")
    cst_d = nc.dram_tensor("cst", [128, 4], mt.float32, kind="ExternalInput")
    coef_d = nc.dram_tensor("coef", [128, 8], mt.float32, kind="ExternalInput")
    out_d = nc.dram_tensor("out", [2, COUT, NBANK, 448], mt.float16,
                           kind="ExternalOutput")

    with tile.TileContext(nc) as tc:
        with tc.tile_pool(name="main", bufs=1) as P, \
             tc.tile_pool(name="psum", bufs=8, space="PSUM") as PS, \
             tc.tile_pool(name="dram", bufs=1, space="DRAM") as D:

            # ---- persistent tiles ----
            xa2f = P.tile([96, PW, PW], mt.float8e4)
            y = P.tile([128, NBANK, 448], mt.float16)
            sums = P.tile([128, NBANK // 2], mt.float32)
            sqs = P.tile([128, NBANK // 2], mt.float32)

            neg1 = P.tile([128, 1], mt.float32)
            nc.vector.memset(neg1[:], -1.0)
            scr1 = P.tile([128, 1], mt.float32)
            nc.vector.memset(scr1[:], 1.0)
            nc.scalar.activation(scr1[:], scr1[:], AF.Sqrt)
            two = P.tile([128, 1], mt.float32)
            nc.vector.memset(two[:], 2.0)

            # top/bottom pads; A row 224 is rewritten by the (h1, s7) scatter
            nc.vector.memset(xa2f[0:96, 0, :], 0.0)
            nc.vector.memset(xa2f[0:96, 224:226, :], 0.0)

            # ---- binarize + scatter, half-chunks (hf, hh, c) of 7 rows ----
            # all loads issued first so the SP queue never head-blocks them
            HQ = RQ // 2            # 1582 = 7*226
            xv = xdev_d.ap().rearrange("p f c (h q) -> p f c h q", h=2)
            chunks = [(hf, hh, c) for hf in range(2) for hh in range(2)
                      for c in range(3)]
            x1s = {}

            def load_chunk(ch):
                hf, hh, c = ch
                x1h = P.tile([128, HQ], mt.float32, tag="x1", bufs=4,
                             name=f"x1_{hf}_{hh}_{c}")
                nc.sync.dma_start(x1h[:], xv[:, hf, c, hh, :])
                x1s[ch] = x1h

            for ch in chunks[:4]:
                load_chunk(ch)

            # constants (issued after the first x loads; needed much later)
            wdr = P.tile([96, 5, 2, 128], mt.float8e4)
            nc.sync.dma_start(
                wdr[:], wdr_d.ap().rearrange("p (d t m) -> p d t m", d=5, t=2))
            cst = P.tile([128, 4], mt.float32)
            nc.sync.dma_start(cst[:], cst_d.ap())
            coef = P.tile([128, 8], mt.float32)
            if general_affine:
                nc.sync.dma_start(coef[:], coef_d.ap())

            for ci, (hf, hh, c) in enumerate(chunks):
                x1h = x1s[(hf, hh, c)]
                if general_affine:
                    nc.vector.tensor_scalar(
                        x1h[:], x1h[:], coef[:, c:c + 1],
                        coef[:, 3 + c:4 + c], AO.mult, AO.add)
                m1 = P.tile([128, HQ], mt.bfloat16, tag="m1", bufs=3,
                            name=f"m1_{hf}_{hh}_{c}")
                rint_eng = nc.vector if ci >= 10 else nc.gpsimd
                rint_eng.tensor_scalar(m1[:], x1h[:], MAGIC, MAGIC,
                                       AO.add, AO.subtract)
                nc.vector.tensor_tensor(m1[:], x1h[:], m1[:], AO.is_ge)
                xa1b = P.tile([128, 7, PW], mt.float8e4, tag="xa1b", bufs=2,
                              name=f"xa1b_{hf}_{hh}_{c}")
                if ci < 2:   # ring pads zeroed once; op3 writes interior only
                    nc.gpsimd.memset(xa1b[:, :, 0], 0.0)
                    nc.gpsimd.memset(xa1b[:, :, 225], 0.0)
                m1v = m1[:].rearrange("p (a b) -> p a b", a=7)
                if ci >= 10:
                    nc.vector.tensor_scalar(xa1b[:, :, 1:225],
                                            m1v[:, :, 1:225], 2.0, 1.0,
                                            AO.mult, AO.subtract)
                else:
                    nc.scalar.activation(xa1b[:, :, 1:225], m1v[:, :, 1:225],
                                         AF.Identity, bias=neg1[:],
                                         scale=two[:])
                # scatter all 8 segs in one DMA (partition p = 8g + s)
                abase = xa2f[16 * c:16 * c + 16, 0:1, 0:1]
                dst = AP(abase.tensor,
                         int(abase.offset) + (14 * hf + 7 * hh + 1) * PW,
                         [[int(abase.ap[0][0]), 16], [28 * PW, SEGS], [1, HQ]])
                nc.sync.dma_start(dst, xa1b[:].rearrange("p a b -> p (a b)"))
                # B-half scatter: B[r] = A[r+1], same source chunk
                bbase = xa2f[48 + 16 * c:64 + 16 * c, 0:1, 0:1]
                bdst = AP(bbase.tensor,
                          int(bbase.offset) + (14 * hf + 7 * hh) * PW,
                          [[int(bbase.ap[0][0]), 16], [28 * PW, SEGS], [1, HQ]])
                nc.sync.dma_start(bdst, xa1b[:].rearrange("p a b -> p (a b)"))
                if ci + 4 < len(chunks):
                    load_chunk(chunks[ci + 4])

            # ---- conv: DoubleRow matmuls ----
            xbase = xa2f[0:96, 0:1, 0:1]
            pstride = int(xbase.ap[0][0])
            xoff = int(xbase.offset)

            perf = mybir.MatmulPerfMode.DoubleRow
            for bp2 in range(NBANK // 2):
                ps = PS.tile([128, 2, 512], mt.float32, tag="ps", bufs=4,
                             name=f"ps_{bp2}")
                for half in range(2):
                    b = 2 * bp2 + half
                    y0 = 4 * b
                    for r in range(2):
                        for d, (t0, t1) in enumerate(DR_PAIRS):
                            o0 = (y0 + r + t0[0]) * PW + t0[1]
                            o1 = (y0 + r + t1[0]) * PW + t1[1]
                            mv = AP(xbase.tensor, xoff + o0,
                                    [[pstride, 96], [o1 - o0, 2], [1, 224]])
                            nc.tensor.matmul(
                                ps[:, half, 224 * r:224 * r + 224],
                                wdr[:, d, :, :], mv,
                                start=(d == 0), stop=(d == 4),
                                perf_mode=perf)
                nc.vector.tensor_scalar(y[:, 2 * bp2:2 * bp2 + 2, :],
                                        ps[:, :, 0:448],
                                        1.0, None, AO.mult, AO.add,
                                        accum_out=sums[:, bp2:bp2 + 1])
                nc.scalar.activation(ps[:, :, 0:448], ps[:, :, 0:448],
                                     AF.Square,
                                     accum_out=sqs[:, bp2:bp2 + 1])

            # ---- bypass loads (fp16, straight into y layout) ----
            byp_tiles = {}

            def load_byp(s):
                bp = P.tile([128, 7, 448], mt.float16, tag="byp", bufs=8,
                            name=f"byp_{s}")
                for ci in range(2):
                    src = AP(xhalf_d.ap().tensor, 6272 * s + 448 * ci,
                             [[H * W, COUT], [896, 7], [1, 448]])
                    nc.sync.dma_start(bp[64 * ci:64 * ci + 64, :, :], src)
                return bp

            for s in range(SEGS):
                byp_tiles[s] = load_byp(s)

            # ---- stats + collective + BN affine (all on 128 partitions) ----
            kc = P.tile([128, 2], mt.float32)
            sums2 = P.tile([128, 2], mt.float32)
            nc.vector.reduce_sum(sums2[:, 0:1], sums[:], axis=mybir.AxisListType.X)
            nc.vector.reduce_sum(sums2[:, 1:2], sqs[:], axis=mybir.AxisListType.X)
            cbin = D.tile([128, 2], mt.float32)
            cbout = D.tile([NCORES, 128, 2], mt.float32)
            nc.scalar.dma_start(cbin[:], sums2[:])
            nc.gpsimd.collective_compute(
                "AllGather", mybir.AluOpType.bypass,
                replica_groups=[list(range(NCORES))],
                ins=[cbin.opt()], outs=[cbout.opt()])
            # gather (core, half) entries onto BOTH partition halves
            gath = P.tile([128, 2, 2 * NCORES], mt.float32)
            cbt = cbout[:].rearrange("g (h p) q -> g h p q", h=2)
            for half in range(2):
                src = AP(cbt.tensor, 0,
                         [[2, 64], [1, 2], [128, 2 * NCORES]])
                nc.sync.dma_start(gath[64 * half:64 * half + 64, :, :], src)
            mv2 = P.tile([128, 2], mt.float32)
            nc.vector.reduce_sum(mv2[:], gath[:], axis=mybir.AxisListType.X)
            nc.vector.tensor_scalar(mv2[:], mv2[:], 1.0 / float(B * H * W),
                                    None, AO.mult)

            m2t = P.tile([128, 1], mt.float32)
            nc.vector.tensor_tensor(m2t[:], mv2[:, 0:1], mv2[:, 0:1], AO.mult)
            vart = P.tile([128, 1], mt.float32)
            nc.vector.tensor_tensor(vart[:], mv2[:, 1:2], m2t[:], AO.subtract)
            t1 = P.tile([128, 1], mt.float32)
            nc.vector.tensor_tensor(t1[:], vart[:], cst[:, 0:1], AO.mult)
            nc.vector.tensor_scalar(t1[:], t1[:], BN_EPS, None, AO.add)
            sq = P.tile([128, 1], mt.float32)
            nc.scalar.activation(sq[:], t1[:], AF.Sqrt)
            rc = P.tile([128, 1], mt.float32)
            nc.vector.reciprocal(rc[:], sq[:])
            nc.vector.tensor_tensor(kc[:, 0:1], rc[:], cst[:, 1:2], AO.mult)
            mk = P.tile([128, 1], mt.float32)
            nc.vector.tensor_tensor(mk[:], mv2[:, 0:1], kc[:, 0:1], AO.mult)
            nc.vector.tensor_tensor(kc[:, 1:2], cst[:, 2:3], mk[:],
                                    AO.subtract)

            # ---- pass 2: affine + bypass + store ----
            for s in range(SEGS):
                bp = byp_tiles.pop(s)
                if s == 0:
                    nc.vector.tensor_scalar(bp[:], bp[:], kc[:, 1:2], None,
                                            AO.add)
                else:
                    nc.scalar.activation(bp[:], bp[:], AF.Identity,
                                         bias=kc[:, 1:2])
                ob = P.tile([128, 7, 448], mt.float16, tag="ob", bufs=3,
                            name=f"ob_{s}")
                nc.vector.scalar_tensor_tensor(
                    ob[:], y[:, 7 * s:7 * s + 7, :],
                    kc[:, 0:1], bp[:], AO.mult, AO.add)
                nc.gpsimd.dma_start(
                    out_d.ap()[:, :, 7 * s:7 * s + 7, :], ob[:])

    nc.compile()
    return nc


def _get_nc(general_affine):
    key = ("nc", general_affine, NCORES)
    if key not in _cache:
        _cache[key] = _build(general_affine)
    return _cache[key]


def _pack_weights(wt):
    """wt [64, 48, 3, 3] (+-1 * A, slot-permuted) -> [96, 5, 2, 128] f32."""
    w = np.zeros((96, 5, 2, 128), np.float32)
    covered = set()
    for d, pair in enumerate(DR_PAIRS):
        for t, tl in enumerate(pair):
            if len(tl) == 3:
                continue
            delta, kw = tl
            for stack in (0, 1):
                for half, rho in ((0, 0), (1, 2)):
                    kh = delta + stack - rho
                    if 0 <= kh <= 2 and (rho, kh, kw) not in covered:
                        covered.add((rho, kh, kw))
                        w[48 * stack:48 * stack + 48, d, t,
                          64 * half:64 * half + 64] = wt[:, :, kh, kw].T
    assert len(covered) == 18
    return w


def _host_prep(alpha, epsilon, tau, A, weight, gamma, beta):
    import ml_dtypes
    f8 = ml_dtypes.float8_e4m3

    eps_v = np.asarray(epsilon, np.float32).reshape(-1)
    tau_v = np.asarray(tau, np.float32).reshape(-1)
    A_v = np.asarray(A, np.float32).reshape(-1)
    if eps_v.size == 1:
        eps_v = np.full(CIN, eps_v[0], np.float32)
    if tau_v.size == 1:
        tau_v = np.full(CIN, tau_v[0], np.float32)
    if A_v.size == 1:
        A_v = np.full(CIN, A_v[0], np.float32)

    general = not (np.all(eps_v == 0.0) and np.all(tau_v == 1.0))

    w = np.asarray(weight, np.float32)
    scale = np.mean(np.abs(w), axis=(1, 2, 3), dtype=np.float32)
    waff = np.sign(w) * A_v[None, :, None, None]
    wperm = waff[:, SLOT_TO_CH, :, :]
    wdr = _pack_weights(wperm).reshape(96, -1).astype(f8)

    cst = np.zeros((64, 4), np.float32)
    cst[:, 0] = scale * scale
    cst[:, 1] = np.asarray(gamma, np.float32).reshape(-1) * scale
    cst[:, 2] = np.asarray(beta, np.float32).reshape(-1)
    cst = np.tile(cst, (2, 1))

    coef = np.zeros((128, 8), np.float32)
    if general:
        for p in range(128):
            g = p // 8
            for c in range(3):
                ch = 45 + c if g == 15 else 15 * c + g
                coef[p, c] = 1.0 / tau_v[ch]
                coef[p, 3 + c] = -eps_v[ch] / tau_v[ch]
    return general, wdr, cst, coef


def _make_xdev(xi):
    """xi [48, 224, 224] f32 -> [128, 2, 3, 3164] (rows padded to 226)."""
    xp = np.zeros((CIN, H, PW), np.float32)
    xp[:, :, 1:225] = xi
    xr = xp.reshape(CIN, SEGS, 2, RQ)       # [ch, seg, hf, 14*226]
    p = np.arange(128)
    g_idx, s_idx = p // 8, p % 8
    out = np.empty((128, 2, 3, RQ), np.float32)
    for c in range(3):
        ch = np.where(g_idx == 15, 45 + c, 15 * c + g_idx)
        out[:, :, c, :] = xr[ch, s_idx, :, :]
    return out


def _make_xhalf(xi):
    """xi [48, 224, 224] f32 -> [64, H*W] fp16 (identity + 16 group means)."""
    xh = np.empty((COUT, H * W), np.float16)
    xh[0:CIN] = xi.reshape(CIN, -1).astype(np.float16)
    xf = xi.reshape(CIN, -1)
    xh[48:63] = xf[0:45].reshape(3, 15, -1).mean(axis=0,
                                                 dtype=np.float32).astype(np.float16)
    xh[63] = xf[45:48].mean(axis=0, dtype=np.float32).astype(np.float16)
    return xh


def kernel(x, alpha, epsilon, tau, A, weight, gamma, beta):
    from concourse import bass_utils

    x = np.asarray(x, np.float32)
    general, wdr, cst, coef = _host_prep(alpha, epsilon, tau, A,
                                         weight, gamma, beta)
    nc = _get_nc(general)

    in_maps = []
    for i in range(NCORES):
        xi = np.ascontiguousarray(x[i])
        in_maps.append({
            "xdev": _make_xdev(xi),
            "xhalf": _make_xhalf(xi),
            "wdr": wdr, "cst": cst, "coef": coef,
        })
    res = bass_utils.run_bass_kernel_spmd(nc, in_maps,
                                          core_ids=list(range(NCORES)))
    out = np.stack([
        res.results[i]["out"].reshape(2, COUT, NBANK, 2, 224)
        .transpose(1, 2, 0, 3, 4).reshape(COUT, H, W)
        for i in range(NCORES)
    ])
    return out.astype(np.float32)


# revision 57
# speedup vs baseline: 1.0785x; 1.0047x over previous
"""Trainium2 Bass kernel for nn_BiDenseConv2d (binarized 3x3 conv + sync-BN + channel bypass).

Shapes (hardcoded): x [8, 48, 224, 224] f32 -> out [8, 64, 224, 224] f32.
Sharding: data-parallel over batch, 1 image per core; BN stats all-reduced
([64,2] f32 AllGather); weights replicated.

Per-core pipeline (phases overlap via Tile dataflow; loads interleaved
4-ahead with the binarize chunks so no queue head-blocks):
  1. binarize, 12 half-chunks [128p=(8g+s), 7 rows x 226]: rint via the fp32
     magic constant (Pool; DVE for the last two), is_ge in-place (DVE),
     affine {1,0}->+-1 fp8 on the row interior (ACT; ring pads zeroed once).
  2. scatter to conv layout xa2f [96, 226, 226] fp8: one merged 8-seg DMA per
     chunk for the A half and one for the B half (B[r] = A[r+1], scattered
     from the same chunk).
  3. conv: fp8 DoubleRow matmuls, M=128 (low 64 out-cols = out rows y0+r,
     high 64 = y0+2+r), N=224, 10 DR per 4-row bank; 18 (rho, kh, kw) taps
     packed into (delta in {0,2,4}) x kw tiles over the 2-row stack, tile-pair
     strides >= 224 (3D non-overlapping moving APs); +-1 acts x +-1 weights
     accumulate exact integer sums in PSUM f32 ([128, 2, 512] bank-aligned
     pairs), evicted to fp16.
  4. BN: per-2-bank sums (DVE evict accum) + sumsq (ACT Square accum);
     AllGather of raw [128, 2] sums; (core, half) entries gathered onto both
     partition halves so k = gamma*s*rsqrt(s^2 var + eps), c = beta - mu*k
     (s = mean|w|) compute on 128 partitions without a broadcast.
  5. bypass: host supplies xhalf fp16 [64, H*W] (48 identity channels + 16
     group means); all 8 seg tiles prefetched into the y layout; pass 2:
     bypc = bp + c (ACT), ob = y*k + bypc (one DVE scalar_tensor_tensor per
     half-seg), stored via Pool SWDGE.

Conv input channel at slot 16c+g is channel 15c+g (g<15) / 45+c (g=15),
folded into the weights host-side. Output layout matches the baseline.
"""
import sys
import numpy as np

sys.path.insert(0, '/opt/trn_rl_repo')

B, CIN, COUT, H, W = 8, 48, 64, 224, 224
NCORES = 8
SEGS, SEGR = 8, 28
PW = 226
RQ = 14 * PW            # 3164 elems per (c, hf) chunk row-block
NBANK = 56
BN_EPS = 1e-5
MAGIC = 12582912.0

_cache = {}

SLOT_TO_CH = np.zeros(48, np.int64)
for _c in range(3):
    for _g in range(16):
        SLOT_TO_CH[16 * _c + _g] = (45 + _c) if _g == 15 else (15 * _c + _g)

# DR tile pairs (delta, kw); 'z' = zero-weight tile (arbitrary in-bounds read)
DR_PAIRS = [((0, 0), (2, 0)),
            ((0, 1), (2, 1)),
            ((0, 2), (4, 0)),
            ((2, 2), (4, 1)),
            ((1, 0, 'z'), (4, 2))]


def _build(general_affine: bool):
    from concourse import bacc, tile, mybir
    from concourse.ap import AP
    mt = mybir.dt
    AO = mybir.AluOpType
    AF = mybir.ActivationFunctionType

    nc = bacc.Bacc("TRN2", target_bir_lowering=False, debug=False,
                   num_devices=NCORES)

    xdev_d = nc.dram_tensor("xdev", [128, 2, 3, RQ], mt.float32,
                            kind="ExternalInput")
    xhalf_d = nc.dram_tensor("xhalf", [COUT, H * W], mt.float16,
                             kind="ExternalInput")
    wdr_d = nc.dram_tensor("wdr", [96, 5 * 2 * 128], mt.float8e4,
                           kind="ExternalInput")
    cst_d = nc.dram_tensor("cst", [128, 4], mt.float32, kind="ExternalInput")
    coef_d = nc.dram_tensor("coef", [128, 8], mt.float32, kind="ExternalInput")
    out_d = nc.dram_tensor("out", [2, COUT, NBANK, 448], mt.float16,
                           kind="ExternalOutput")

    with tile.TileContext(nc) as tc:
        with tc.tile_pool(name="main", bufs=1) as P, \
             tc.tile_pool(name="psum", bufs=8, space="PSUM") as PS, \
             tc.tile_pool(name="dram", bufs=1, space="DRAM") as D:

            # ---- persistent tiles ----
            xa2f = P.tile([96, PW, PW], mt.float8e4)
            y = P.tile([128, NBANK, 448], mt.float16)
            sums = P.tile([128, NBANK // 2], mt.float32)
            sqs = P.tile([128, NBANK // 2], mt.float32)

            neg1 = P.tile([128, 1], mt.float32)
            nc.vector.memset(neg1[:], -1.0)
            scr1 = P.tile([128, 1], mt.float32)
            nc.vector.memset(scr1[:], 1.0)
            nc.scalar.activation(scr1[:], scr1[:], AF.Sqrt)
            two = P.tile([128, 1], mt.float32)
            nc.vector.memset(two[:], 2.0)

            # top/bottom pads; A row 224 is rewritten by the (h1, s7) scatter
            nc.vector.memset(xa2f[0:96, 0, :], 0.0)
            nc.vector.memset(xa2f[0:96, 224:226, :], 0.0)

            # ---- binarize + scatter, half-chunks (hf, hh, c) of 7 rows ----
            # all loads issued first so the SP queue never head-blocks them
            HQ = RQ // 2            # 1582 = 7*226
            xv = xdev_d.ap().rearrange("p f c (h q) -> p f c h q", h=2)
            chunks = [(hf, hh, c) for hf in range(2) for hh in range(2)
                      for c in range(3)]
            x1s = {}

            def load_chunk(ch):
                hf, hh, c = ch
                x1h = P.tile([128, HQ], mt.float32, tag="x1", bufs=4,
                             name=f"x1_{hf}_{hh}_{c}")
                nc.sync.dma_start(x1h[:], xv[:, hf, c, hh, :])
                x1s[ch] = x1h

            for ch in chunks[:4]:
                load_chunk(ch)

            # constants (issued after the first x loads; needed much later)
            wdr = P.tile([96, 5, 2, 128], mt.float8e4)
            nc.sync.dma_start(
                wdr[:], wdr_d.ap().rearrange("p (d t m) -> p d t m", d=5, t=2))
            cst = P.tile([128, 4], mt.float32)
            nc.sync.dma_start(cst[:], cst_d.ap())
            coef = P.tile([128, 8], mt.float32)
            if general_affine:
                nc.sync.dma_start(coef[:], coef_d.ap())

            for ci, (hf, hh, c) in enumerate(chunks):
                x1h = x1s[(hf, hh, c)]
                if general_affine:
                    nc.vector.tensor_scalar(
                        x1h[:], x1h[:], coef[:, c:c + 1],
                        coef[:, 3 + c:4 + c], AO.mult, AO.add)
                m1 = P.tile([128, HQ], mt.bfloat16, tag="m1", bufs=3,
                            name=f"m1_{hf}_{hh}_{c}")
                rint_eng = nc.vector if ci >= 10 else nc.gpsimd
                rint_eng.tensor_scalar(m1[:], x1h[:], MAGIC, MAGIC,
                                       AO.add, AO.subtract)
                nc.vector.tensor_tensor(m1[:], x1h[:], m1[:], AO.is_ge)
                xa1b = P.tile([128, 7, PW], mt.float8e4, tag="xa1b", bufs=2,
                              name=f"xa1b_{hf}_{hh}_{c}")
                if ci < 2:   # ring pads zeroed once; op3 writes interior only
                    nc.gpsimd.memset(xa1b[:, :, 0], 0.0)
                    nc.gpsimd.memset(xa1b[:, :, 225], 0.0)
                m1v = m1[:].rearrange("p (a b) -> p a b", a=7)
                if ci >= 10:
                    nc.vector.tensor_scalar(xa1b[:, :, 1:225],
                                            m1v[:, :, 1:225], 2.0, 1.0,
                                            AO.mult, AO.subtract)
                else:
                    nc.scalar.activation(xa1b[:, :, 1:225], m1v[:, :, 1:225],
                                         AF.Identity, bias=neg1[:],
                                         scale=two[:])
                # scatter all 8 segs in one DMA (partition p = 8g + s)
                abase = xa2f[16 * c:16 * c + 16, 0:1, 0:1]
                dst = AP(abase.tensor,
                         int(abase.offset) + (14 * hf + 7 * hh + 1) * PW,
                         [[int(abase.ap[0][0]), 16], [28 * PW, SEGS], [1, HQ]])
                nc.sync.dma_start(dst, xa1b[:].rearrange("p a b -> p (a b)"))
                # B-half scatter: B[r] = A[r+1], same source chunk
                bbase = xa2f[48 + 16 * c:64 + 16 * c, 0:1, 0:1]
                bdst = AP(bbase.tensor,
                          int(bbase.offset) + (14 * hf + 7 * hh) * PW,
                          [[int(bbase.ap[0][0]), 16], [28 * PW, SEGS], [1, HQ]])
                nc.sync.dma_start(bdst, xa1b[:].rearrange("p a b -> p (a b)"))
                if ci + 4 < len(chunks):
                    load_chunk(chunks[ci + 4])

            # ---- conv: DoubleRow matmuls ----
            xbase = xa2f[0:96, 0:1, 0:1]
            pstride = int(xbase.ap[0][0])
            xoff = int(xbase.offset)

            perf = mybir.MatmulPerfMode.DoubleRow
            for bp2 in range(NBANK // 2):
                ps = PS.tile([128, 2, 512], mt.float32, tag="ps", bufs=4,
                             name=f"ps_{bp2}")
                for half in range(2):
                    b = 2 * bp2 + half
                    y0 = 4 * b
                    for r in range(2):
                        for d, (t0, t1) in enumerate(DR_PAIRS):
                            o0 = (y0 + r + t0[0]) * PW + t0[1]
                            o1 = (y0 + r + t1[0]) * PW + t1[1]
                            mv = AP(xbase.tensor, xoff + o0,
                                    [[pstride, 96], [o1 - o0, 2], [1, 224]])
                            nc.tensor.matmul(
                                ps[:, half, 224 * r:224 * r + 224],
                                wdr[:, d, :, :], mv,
                                start=(d == 0), stop=(d == 4),
                                perf_mode=perf)
                nc.vector.tensor_scalar(y[:, 2 * bp2:2 * bp2 + 2, :],
                                        ps[:, :, 0:448],
                                        1.0, None, AO.mult, AO.add,
                                        accum_out=sums[:, bp2:bp2 + 1])
                nc.scalar.activation(ps[:, :, 0:448], ps[:, :, 0:448],
                                     AF.Square,
                                     accum_out=sqs[:, bp2:bp2 + 1])

            # ---- bypass loads (fp16, straight into y layout) ----
            byp_tiles = {}

            def load_byp(s):
                bp = P.tile([128, 7, 448], mt.float16, tag="byp", bufs=8,
                            name=f"byp_{s}")
                for ci in range(2):
                    src = AP(xhalf_d.ap().tensor, 6272 * s + 448 * ci,
                             [[H * W, COUT], [896, 7], [1, 448]])
                    nc.sync.dma_start(bp[64 * ci:64 * ci + 64, :, :], src)
                return bp

            for s in range(SEGS):
                byp_tiles[s] = load_byp(s)

            # ---- stats + collective + BN affine (all on 128 partitions) ----
            kc = P.tile([128, 2], mt.float32)
            sums2 = P.tile([128, 2], mt.float32)
            nc.vector.reduce_sum(sums2[:, 0:1], sums[:], axis=mybir.AxisListType.X)
            nc.vector.reduce_sum(sums2[:, 1:2], sqs[:], axis=mybir.AxisListType.X)
            cbin = D.tile([128, 2], mt.float32)
            cbout = D.tile([NCORES, 128, 2], mt.float32)
            nc.scalar.dma_start(cbin[:], sums2[:])
            nc.gpsimd.collective_compute(
                "AllGather", mybir.AluOpType.bypass,
                replica_groups=[list(range(NCORES))],
                ins=[cbin.opt()], outs=[cbout.opt()])
            # gather (core, half) entries onto BOTH partition halves
            gath = P.tile([128, 2, 2 * NCORES], mt.float32)
            cbt = cbout[:].rearrange("g (h p) q -> g h p q", h=2)
            for half in range(2):
                src = AP(cbt.tensor, 0,
                         [[2, 64], [1, 2], [128, 2 * NCORES]])
                nc.sync.dma_start(gath[64 * half:64 * half + 64, :, :], src)
            mv2 = P.tile([128, 2], mt.float32)
            nc.vector.reduce_sum(mv2[:], gath[:], axis=mybir.AxisListType.X)
            nc.vector.tensor_scalar(mv2[:], mv2[:], 1.0 / float(B * H * W),
                                    None, AO.mult)

            m2t = P.tile([128, 1], mt.float32)
            nc.vector.tensor_tensor(m2t[:], mv2[:, 0:1], mv2[:, 0:1], AO.mult)
            vart = P.tile([128, 1], mt.float32)
            nc.vector.tensor_tensor(vart[:], mv2[:, 1:2], m2t[:], AO.subtract)
            t1 = P.tile([128, 1], mt.float32)
            nc.vector.tensor_tensor(t1[:], vart[:], cst[:, 0:1], AO.mult)
            nc.vector.tensor_scalar(t1[:], t1[:], BN_EPS, None, AO.add)
            sq = P.tile([128, 1], mt.float32)
            nc.scalar.activation(sq[:], t1[:], AF.Sqrt)
            rc = P.tile([128, 1], mt.float32)
            nc.vector.reciprocal(rc[:], sq[:])
            nc.vector.tensor_tensor(kc[:, 0:1], rc[:], cst[:, 1:2], AO.mult)
            mk = P.tile([128, 1], mt.float32)
            nc.vector.tensor_tensor(mk[:], mv2[:, 0:1], kc[:, 0:1], AO.mult)
            nc.vector.tensor_tensor(kc[:, 1:2], cst[:, 2:3], mk[:],
                                    AO.subtract)

            # ---- pass 2: affine + bypass + store ----
            for s in range(SEGS):
                bp = byp_tiles.pop(s)
                ob = P.tile([128, 7, 448], mt.float16, tag="ob", bufs=3,
                            name=f"ob_{s}")
                nc.scalar.activation(ob[:], y[:, 7 * s:7 * s + 7, :],
                                     AF.Identity, bias=kc[:, 1:2],
                                     scale=kc[:, 0:1])
                nc.vector.tensor_tensor(ob[:], ob[:], bp[:], AO.add)
                nc.gpsimd.dma_start(
                    out_d.ap()[:, :, 7 * s:7 * s + 7, :], ob[:])

    nc.compile()
    return nc


def _get_nc(general_affine):
    key = ("nc", general_affine, NCORES)
    if key not in _cache:
        _cache[key] = _build(general_affine)
    return _cache[key]


def _pack_weights(wt):
    """wt [64, 48, 3, 3] (+-1 * A, slot-permuted) -> [96, 5, 2, 128] f32."""
    w = np.zeros((96, 5, 2, 128), np.float32)
    covered = set()
    for d, pair in enumerate(DR_PAIRS):
        for t, tl in enumerate(pair):
            if len(tl) == 3:
                continue
            delta, kw = tl
            for stack in (0, 1):
                for half, rho in ((0, 0), (1, 2)):
                    kh = delta + stack - rho
                    if 0 <= kh <= 2 and (rho, kh, kw) not in covered:
                        covered.add((rho, kh, kw))
                        w[48 * stack:48 * stack + 48, d, t,
                          64 * half:64 * half + 64] = wt[:, :, kh, kw].T
    assert len(covered) == 18
    return w


def _host_prep(alpha, epsilon, tau, A, weight, gamma, beta):
    import ml_dtypes
    f8 = ml_dtypes.float8_e4m3

    eps_v = np.asarray(epsilon, np.float32).reshape(-1)
    tau_v = np.asarray(tau, np.float32).reshape(-1)
    A_v = np.asarray(A, np.float32).reshape(-1)
    if eps_v.size == 1:
        eps_v = np.full(CIN, eps_v[0], np.float32)
    if tau_v.size == 1:
        tau_v = np.full(CIN, tau_v[0], np.float32)
    if A_v.size == 1:
        A_v = np.full(CIN, A_v[0], np.float32)

    general = not (np.all(eps_v == 0.0) and np.all(tau_v == 1.0))

    w = np.asarray(weight, np.float32)
    scale = np.mean(np.abs(w), axis=(1, 2, 3), dtype=np.float32)
    waff = np.sign(w) * A_v[None, :, None, None]
    wperm = waff[:, SLOT_TO_CH, :, :]
    wdr = _pack_weights(wperm).reshape(96, -1).astype(f8)

    cst = np.zeros((64, 4), np.float32)
    cst[:, 0] = scale * scale
    cst[:, 1] = np.asarray(gamma, np.float32).reshape(-1) * scale
    cst[:, 2] = np.asarray(beta, np.float32).reshape(-1)
    cst = np.tile(cst, (2, 1))

    coef = np.zeros((128, 8), np.float32)
    if general:
        for p in range(128):
            g = p // 8
            for c in range(3):
                ch = 45 + c if g == 15 else 15 * c + g
                coef[p, c] = 1.0 / tau_v[ch]
                coef[p, 3 + c] = -eps_v[ch] / tau_v[ch]
    return general, wdr, cst, coef


def _make_xdev(xi):
    """xi [48, 224, 224] f32 -> [128, 2, 3, 3164] (rows padded to 226)."""
    xp = np.zeros((CIN, H, PW), np.float32)
    xp[:, :, 1:225] = xi
    xr = xp.reshape(CIN, SEGS, 2, RQ)       # [ch, seg, hf, 14*226]
    p = np.arange(128)
    g_idx, s_idx = p // 8, p % 8
    out = np.empty((128, 2, 3, RQ), np.float32)
    for c in range(3):
        ch = np.where(g_idx == 15, 45 + c, 15 * c + g_idx)
        out[:, :, c, :] = xr[ch, s_idx, :, :]
    return out


def _make_xhalf(xi):
    """xi [48, 224, 224] f32 -> [64, H*W] fp16 (identity + 16 group means)."""
    xh = np.empty((COUT, H * W), np.float16)
    xh[0:CIN] = xi.reshape(CIN, -1).astype(np.float16)
    xf = xi.reshape(CIN, -1)
    xh[48:63] = xf[0:45].reshape(3, 15, -1).mean(axis=0,
                                                 dtype=np.float32).astype(np.float16)
    xh[63] = xf[45:48].mean(axis=0, dtype=np.float32).astype(np.float16)
    return xh


def kernel(x, alpha, epsilon, tau, A, weight, gamma, beta):
    from concourse import bass_utils

    x = np.asarray(x, np.float32)
    general, wdr, cst, coef = _host_prep(alpha, epsilon, tau, A,
                                         weight, gamma, beta)
    nc = _get_nc(general)

    in_maps = []
    for i in range(NCORES):
        xi = np.ascontiguousarray(x[i])
        in_maps.append({
            "xdev": _make_xdev(xi),
            "xhalf": _make_xhalf(xi),
            "wdr": wdr, "cst": cst, "coef": coef,
        })
    res = bass_utils.run_bass_kernel_spmd(nc, in_maps,
                                          core_ids=list(range(NCORES)))
    out = np.stack([
        res.results[i]["out"].reshape(2, COUT, NBANK, 2, 224)
        .transpose(1, 2, 0, 3, 4).reshape(COUT, H, W)
        for i in range(NCORES)
    ])
    return out.astype(np.float32)
